# revision 1
# baseline (speedup 1.0000x reference)
"""Trainium2 Bass kernel: MHA with 1.5-entmax, head-averaged attention map.

Problem (hardcoded): B=2, L=1536, D=768, H=12, dk=64.
  q = query @ wq_w.T + wq_b ; k = key @ wk_w.T + wk_b
  scores = q.k/sqrt(dk) per head, key-padding mask -> -1e9
  p = entmax15(scores); out = mean_h p -> [B, Lq, Lk]

Algorithm (per 128-query-row x LK tile, all row passes on compacted keys):
 * Host key compaction: masked key columns give exactly p=0 (their z=0 sits
   below tau* >= 0.15), so the host gathers unmasked keys, pads to LK
   (multiple of 64, >= 768), and scatters the kernel output back into a
   zero [B, L, L]. Lossless; shrinks every row pass ~1.8x.
 * q/k/weights are rounded to bf16 (host side for q/k/w); projections run
   bf16 x bf16 on PE into PSUM f32; QpT/KpT are stored f32r so the score
   matmuls stay near-f32 precise.
 * tau solve on r0 = bf16(relu(z - TAU0)) (valid since TAU0 < min-row tau*;
   relu(z - t) == relu(r0 - (t - TAU0)) for t >= TAU0):
     eval0: the PSUM->SBUF copy is an ACT relu with bias, accumulating
            g0 = sum r0; k0 is approximated by 0.327 * (#unmasked keys)
            (one setup reduction off the mask vector).
     quad1: fixed-support solve with modeled f0 = 2 g0^2/k0
            -> t1 = 1.6 * (g0 - sqrt(relu(k0 - g0^2)))/k0
     eval1: r1 (TSP relu), g1 (TSP sum), k1 (TSP is_gt count) - all 4x-mode
            DVE ops - and f1 = sum r1^2 (ACT square / DVE custom, split
            across heads to balance engines).
     quad2: exact fixed-support solve from fresh (f1, g1, k1).
     final: p = bf16(relu(r0 - t2)^2) with accumulated s = sum p via the
            custom DVE op (ACT relu+square for 2 of 6 heads).
 * Normalization: out accumulates p * 1/(s*H) per head as a PE
   diagonal-matmul into PSUM f32 - row sums match entmax's sum=1 exactly,
   cancelling the first-order tau error; costs no vector-engine time.
 * Engines: ACT relu-copies + squares + sqrt; DVE TSPs/customs/recips;
   Pool the tau small-op chains + diag builds; PE scores + accumulation.

Sharding: 8 cores x 384 contiguous query rows (cores 0-3 batch 0, cores
4-7 batch 1); all 12 heads local to each core, no collectives.
"""

import math

import numpy as np

import concourse.bass as bass
import concourse.tile as tile
from concourse import bacc, mybir
from concourse.masks import make_identity


def _register_entmax_op():
    """Custom DVE op: out = relu(in0 + c0)^2, accum = c1 + sum(out)."""
    import numpy as np
    from operator import add as _op_add

    from concourse import dve_ops
    from concourse.dve_spec import C0, C1, Spec, Src0, lower, relu, sq
    from concourse.dve_uop import DveOpSpec

    name = "ENTMAX_SQACC_ANT"
    for o in dve_ops.OPS:
        if o.name == name:
            return o

    def _ref(in0, in1, c0, c1, c2):
        P = in0.shape[0]
        x = in0.astype(np.float32).reshape(P, -1)
        c0v = c0 if isinstance(c0, (int, float)) else np.asarray(c0, np.float32).reshape(-1, 1)
        c1v = c1 if isinstance(c1, (int, float)) else np.asarray(c1, np.float32).reshape(-1, 1)
        body = np.maximum(np.nan_to_num(x + c0v, nan=0.0, posinf=np.inf, neginf=-np.inf), 0) ** 2
        acc = c1v + body.sum(-1, keepdims=True)
        return body, acc

    spec = Spec(body=sq(relu(Src0 + C0)), accum=_op_add, accum_init=C1,
                reference=_ref)
    row = max(dve_ops._SUB_OPCODE_FOR_NAME.values()) + 1
    assert row < 0x20
    dve_ops._SUB_OPCODE_FOR_NAME[name] = row
    shas = {}
    for ver in ("v3", "v4"):
        u = lower(spec, ver=ver)
        shas[ver] = DveOpSpec(name=name, opcode=row, uops=u, rd1_en=False).sha(ver)
    op = dve_ops.DveOp(name, spec, subdim=False, uops_sha=shas)
    dve_ops.OPS.append(op)
    dve_ops.CUSTOM_DVE_SPECS[name] = spec
    return op


ENTMAX_OP = _register_entmax_op()

F32 = mybir.dt.float32
FR = mybir.dt.float32r
BF16 = mybir.dt.bfloat16
AL = mybir.AluOpType
AF = mybir.ActivationFunctionType

B, L, D = 2, 1536, 768
H, DK = 12, 64
P = 128
CORES = 8
QS = (B * L) // CORES          # 384 query rows per core
NT = QS // P                   # 3 q-subtiles per core
KD = D // P                    # 6 contraction chunks of 128
LK_DEFAULT = 768               # compacted+padded key columns (multiple of 128)
SCALE = 1.0 / (2.0 * math.sqrt(DK))   # fold entmax /2 into score scale
TAU0 = 0.10
OM1 = 1.6
CLAMP_HI = 1.2
GH = 6                         # heads per tau group
NGRP = H // GH                 # 2 groups per q-tile
ACT_F1 = (1, 3, 5)             # heads whose f1-eval runs on ACT (rest DVE)
ACT_F2 = (0, 1, 3, 5)          # heads whose f2-eval runs on ACT (rest DVE)
ACT_FINAL = (0, 4)             # heads whose final runs on ACT (rest DVE)
EPS = 1e-20


def _build_program(LK):
    KTF = LK // P                  # full 128-row key tiles
    KREM = LK - KTF * P            # 0 or 64 remainder rows
    CCS = (512, LK - 512)          # score col chunks (both >=256 for f32r)
    CCO = (0, 512)
    nc = bacc.Bacc("TRN2", debug=False)

    q_d = nc.dram_tensor("q_slice", [QS, D], BF16, kind="ExternalInput")
    k_d = nc.dram_tensor("key_c", [LK, D], BF16, kind="ExternalInput")
    wqT_d = nc.dram_tensor("wqT", [D, D], BF16, kind="ExternalInput")
    wkT_d = nc.dram_tensor("wkT", [D, D], BF16, kind="ExternalInput")
    bq_d = nc.dram_tensor("bq", [D], F32, kind="ExternalInput")
    bk_d = nc.dram_tensor("bk", [D], F32, kind="ExternalInput")
    mm_d = nc.dram_tensor("maskmul", [LK], F32, kind="ExternalInput")
    out_d = nc.dram_tensor("out_slice", [QS, LK], F32, kind="ExternalOutput")

    with tile.TileContext(nc) as tc:
        from contextlib import ExitStack

        with ExitStack() as ctx:
            consts = ctx.enter_context(tc.tile_pool(name="consts", bufs=1))
            proj = ctx.enter_context(tc.tile_pool(name="proj", bufs=1))
            ldk = ctx.enter_context(tc.tile_pool(name="ldk", bufs=1))
            ld = ctx.enter_context(tc.tile_pool(name="ld", bufs=1))
            spsum = ctx.enter_context(
                tc.tile_pool(name="spsum", bufs=2, space="PSUM"))

            ident = consts.tile([P, P], BF16)
            make_identity(nc, ident)
            ntau0 = consts.tile([P, 1], F32)
            nc.vector.memset(ntau0, -TAU0)

            maskb = consts.tile([P, LK], F32)
            mm_ap = mm_d[:]
            mm_bcast = bass.AP(
                tensor=mm_ap.tensor, offset=mm_ap.offset,
                ap=[[0, P]] + list(mm_ap.ap),
            )
            nc.sync.dma_start(out=maskb, in_=mm_bcast)

            bq_sb = consts.tile([P, KD], F32)
            nc.sync.dma_start(out=bq_sb, in_=bq_d[:].rearrange("(a p) -> p a", p=P))
            bk_sb = consts.tile([P, KD], F32)
            nc.sync.dma_start(out=bk_sb, in_=bk_d[:].rearrange("(a p) -> p a", p=P))

            k0c = consts.tile([P, 1], F32)
            scrm = consts.tile([P, LK], F32)
            nc.vector.tensor_scalar(scrm, maskb, 0.327, 0.0, AL.mult,
                                    AL.add, accum_out=k0c)
            nc.gpsimd.tensor_scalar(k0c, k0c, 1.0, None, AL.max)
            rk0c = consts.tile([P, 1], F32)
            nc.vector.reciprocal(rk0c, k0c)

            QpT = [proj.tile([P, QS], FR, tag=f"qpt{m}", name=f"QpT{m}") for m in range(KD)]
            KpT = [proj.tile([P, LK], FR, tag=f"kpt{m}", name=f"KpT{m}") for m in range(KD)]

            # ------------- setup: loads, transposes, projections -------------
            q_re = q_d[:].rearrange("(a p) n -> p a n", p=P)
            q_tmp = [ld.tile([P, D], BF16, tag=f"qt{a}", name=f"q_tmp{a}") for a in range(NT)]
            for a in range(NT):
                nc.sync.dma_start(out=q_tmp[a], in_=q_re[:, a, :])
            k_re = k_d[:KTF * P].rearrange("(a p) n -> p a n", p=P)
            k_tmp = [ld.tile([P, D], BF16, tag=f"kt{a}", name=f"k_tmp{a}") for a in range(KTF)]
            for a in range(KTF):
                nc.sync.dma_start(out=k_tmp[a], in_=k_re[:, a, :])
            if KREM:
                k_half = ld.tile([KREM, D], BF16, tag="kth", name="k_half")
                nc.sync.dma_start(out=k_half, in_=k_d[KTF * P:])

            wq_b = [ld.tile([P, D], BF16, tag=f"wq{a}", name=f"wq_b{a}") for a in range(KD)]
            wk_b = [ld.tile([P, D], BF16, tag=f"wk{a}", name=f"wk_b{a}") for a in range(KD)]
            wq_re = wqT_d[:].rearrange("(a p) n -> p a n", p=P)
            wk_re = wkT_d[:].rearrange("(a p) n -> p a n", p=P)
            for a in range(KD):
                nc.sync.dma_start(out=wk_b[a], in_=wk_re[:, a, :])
            for a in range(KD):
                nc.sync.dma_start(out=wq_b[a], in_=wq_re[:, a, :])

            qT = [ldk.tile([P, QS], BF16, tag=f"qTj{j}", name=f"qT{j}") for j in range(KD)]
            kT = [ldk.tile([P, LK], BF16, tag=f"kTj{j}", name=f"kT{j}") for j in range(KD)]

            # q transposes: [128q,128din] blocks -> qT[j][:, a*128] (bf16)
            for j in range(KD):
                pt = spsum.tile([P, QS], BF16, tag="pt", name=f"ptq{j}")
                for a in range(NT):
                    nc.tensor.transpose(
                        pt[:, a * P:(a + 1) * P],
                        q_tmp[a][:, j * P:(j + 1) * P], ident)
                nc.vector.tensor_copy(qT[j], pt)
            # k transposes -> kT[j] (bf16) in 512/384 chunks
            for j in range(KD):
                for cw, co in zip(CCS, CCO):
                    pt = spsum.tile([P, cw], BF16, tag="pt", name=f"ptk{j}_{co}")
                    off = 0
                    while off < cw:
                        a = (co + off) // P
                        if a < KTF:
                            nc.tensor.transpose(
                                pt[:, off:off + P],
                                k_tmp[a][:, j * P:(j + 1) * P], ident)
                            off += P
                        else:
                            nc.tensor.transpose(
                                pt[:, off:off + KREM],
                                k_half[:, j * P:(j + 1) * P],
                                ident[:KREM, :KREM])
                            off += KREM
                    nc.vector.tensor_copy(kT[j][:, co:co + cw], pt)

            # projections (bf16 x bf16 -> psum f32 -> f32r with bias/mask),
            # emitted per dout-chunk m so first-round scores interleave
            def emit_projection(m):
                pq = spsum.tile([P, QS], F32, tag="pt", name=f"pq{m}")
                for kk in range(KD):
                    nc.tensor.matmul(
                        pq, wq_b[kk][:, m * P:(m + 1) * P], qT[kk],
                        start=(kk == 0), stop=(kk == KD - 1))
                nc.vector.tensor_scalar(
                    QpT[m], pq, bq_sb[:, m:m + 1], SCALE,
                    AL.add, AL.mult)
                for cw, co in zip(CCS, CCO):
                    pk = spsum.tile([P, cw], F32, tag="pt", name=f"pk{m}_{co}")
                    for kk in range(KD):
                        nc.tensor.matmul(
                            pk, wk_b[kk][:, m * P:(m + 1) * P],
                            kT[kk][:, co:co + cw],
                            start=(kk == 0), stop=(kk == KD - 1))
                    nc.vector.scalar_tensor_tensor(
                        KpT[m][:, co:co + cw], pk,
                        bk_sb[:, m:m + 1], maskb[:, co:co + cw],
                        AL.add, AL.mult)

            # ---------------- main loop ----------------
            r0p = ctx.enter_context(tc.tile_pool(name="r0p", bufs=3 * GH + 2))
            r1p = ctx.enter_context(tc.tile_pool(name="r1p", bufs=GH + 6))
            pp = ctx.enter_context(tc.tile_pool(name="pp", bufs=GH + 4))
            scrp = ctx.enter_context(tc.tile_pool(name="scrp", bufs=12))
            dgp = ctx.enter_context(tc.tile_pool(name="dgp", bufs=6))
            sm = ctx.enter_context(tc.tile_pool(name="sm", bufs=8))
            accsb = ctx.enter_context(tc.tile_pool(name="accsb", bufs=2))
            zpsum = ctx.enter_context(
                tc.tile_pool(name="zpsum", bufs=2, space="PSUM"))
            apsum = ctx.enter_context(
                tc.tile_pool(name="apsum", bufs=1, space="PSUM"))

            accs = {}
            state = {}

            def emit_scores_eval0(rd, js=None):
                t, grp = rd
                if js is None:
                    js = range(GH)
                if rd not in state:
                    if grp == 0:
                        accs[t] = apsum.tile([P, LK], F32, tag="acc",
                                             name=f"acc{t}")
                    st = state[rd] = {}
                    st["r0"] = []
                    st["g0"] = sm.tile([P, GH], F32, tag="g0",
                                       name=f"g0_{t}_{grp}")
                st = state[rd]
                r0s = st["r0"]
                g0 = st["g0"]
                for j in js:
                    h = grp * GH + j
                    mt, po = h // 2, (h % 2) * DK
                    z = zpsum.tile([P, LK], F32, tag="z",
                                   name=f"z{t}_{grp}_{j}")
                    for cw, co in zip(CCS, CCO):
                        nc.tensor.matmul(
                            z[:, co:co + cw],
                            QpT[mt][po:po + DK, t * P:(t + 1) * P],
                            KpT[mt][po:po + DK, co:co + cw],
                            start=True, stop=True)
                    r0 = r0p.tile([P, LK], BF16, tag="r0",
                                  name=f"r0_{t}_{grp}_{j}")
                    r0s.append(r0)
                    nc.scalar.activation(r0, z, AF.Relu, bias=ntau0[:, 0:1],
                                         scale=1.0,
                                         accum_out=g0[:, j:j + 1])

            def emit_quad1(rd):
                # modeled f0 = 2 g0^2/k0: x = (g0 - sqrt(relu(k0 - g0^2)))/k0
                t, grp = rd
                st = state[rd]
                pref = f"q1_{t}_{grp}"
                g0 = st["g0"]
                gg = sm.tile([P, GH], F32, tag="gg0", name=f"{pref}gg")
                nc.gpsimd.tensor_tensor(gg, g0, g0, AL.mult)
                disc = sm.tile([P, GH], F32, tag="d0", name=f"{pref}disc")
                nc.gpsimd.tensor_tensor(disc, k0c[:, 0:1].to_broadcast([P, GH]),
                                        gg, AL.subtract)
                nc.gpsimd.tensor_scalar(disc, disc, 0.0, None, AL.max)
                sq = sm.tile([P, GH], F32, tag="sq0", name=f"{pref}sq")
                nc.scalar.activation(sq, disc, AF.Sqrt)
                num = sm.tile([P, GH], F32, tag="n0", name=f"{pref}num")
                nc.gpsimd.tensor_tensor(num, g0, sq, AL.subtract)
                x = sm.tile([P, GH], F32, tag="x0", name=f"{pref}x")
                nc.gpsimd.tensor_tensor(x, num,
                                        rk0c[:, 0:1].to_broadcast([P, GH]),
                                        AL.mult)
                nt1 = sm.tile([P, GH], F32, tag="nt1", name=f"{pref}nt1")
                nc.gpsimd.tensor_scalar(nt1, x, -OM1, None, AL.mult)
                nc.gpsimd.tensor_scalar(nt1, nt1, -CLAMP_HI, 0.0, AL.max,
                                        AL.min)
                st["nt1"] = nt1

            def emit_eval1(rd):
                t, grp = rd
                act_f1 = ACT_F1 if rounds.index(rd) >= 2 else (1,)
                st = state[rd]
                nt1 = st["nt1"]
                g1 = sm.tile([P, GH], F32, tag="g1", name=f"g1_{t}_{grp}")
                k1 = sm.tile([P, GH], F32, tag="k1", name=f"k1_{t}_{grp}")
                f1 = sm.tile([P, GH], F32, tag="f1", name=f"f1_{t}_{grp}")
                st["g1"], st["k1"], st["f1"] = g1, k1, f1
                st["r1"] = []
                for j in range(GH):
                    r0 = st["r0"][j]
                    r1 = r1p.tile([P, LK], BF16, tag="r1",
                                  name=f"r1_{t}_{grp}_{j}")
                    st["r1"].append(r1)
                    nc.vector.tensor_scalar(r1, r0, nt1[:, j:j + 1], 0.0,
                                            AL.add, AL.max)
                    scr = scrp.tile([P, LK], BF16, tag="scr", name="gscr1")
                    nc.vector.tensor_scalar(scr, r1, 0.0, 0.0, AL.add,
                                            AL.add, accum_out=g1[:, j:j + 1])
                    scrk = scrp.tile([P, LK], BF16, tag="scr", name="kscr1")
                    nc.vector.tensor_scalar(scrk, r1, 0.0, 0.0, AL.is_gt,
                                            AL.add, accum_out=k1[:, j:j + 1])
                    if j in act_f1:
                        scr3 = scrp.tile([P, LK], BF16, tag="scr",
                                         name="fscr1")
                        nc.scalar.activation(scr3, r1, AF.Square,
                                             accum_out=f1[:, j:j + 1])
                    else:
                        scr3 = scrp.tile([P, LK], BF16, tag="scr",
                                         name="fscr1d")
                        nc.vector._custom_dve(
                            ENTMAX_OP, out=scr3, in0=r0,
                            s0=nt1[:, j:j + 1], s1=0.0,
                            accum_out=f1[:, j:j + 1])

            def emit_quad2(rd):
                t, grp = rd
                st = state[rd]
                pref = f"q2_{t}_{grp}"
                f1, g1 = st["f1"], st["g1"]
                gg = sm.tile([P, GH], F32, tag="gg1", name=f"{pref}gg")
                nc.gpsimd.tensor_tensor(gg, g1, g1, AL.mult)
                ks = sm.tile([P, GH], F32, tag="ks1", name=f"{pref}ks")
                nc.gpsimd.tensor_scalar(ks, st["k1"], 1.0, None, AL.max)
                fm = sm.tile([P, GH], F32, tag="fm1", name=f"{pref}fm")
                nc.gpsimd.tensor_scalar(fm, f1, -1.0, None, AL.add)
                kf = sm.tile([P, GH], F32, tag="kf1", name=f"{pref}kf")
                nc.gpsimd.tensor_tensor(kf, ks, fm, AL.mult)
                disc = sm.tile([P, GH], F32, tag="d1", name=f"{pref}disc")
                nc.gpsimd.tensor_tensor(disc, gg, kf, AL.subtract)
                nc.gpsimd.tensor_scalar(disc, disc, 0.0, None, AL.max)
                sq = sm.tile([P, GH], F32, tag="sq1", name=f"{pref}sq")
                nc.scalar.activation(sq, disc, AF.Sqrt)
                num = sm.tile([P, GH], F32, tag="n1", name=f"{pref}num")
                nc.gpsimd.tensor_tensor(num, g1, sq, AL.subtract)
                rk = sm.tile([P, GH], F32, tag="rk1", name=f"{pref}rk")
                nc.vector.reciprocal(rk, ks)
                x = sm.tile([P, GH], F32, tag="x1", name=f"{pref}x")
                nc.gpsimd.tensor_tensor(x, num, rk, AL.mult)
                nt2 = sm.tile([P, GH], F32, tag="nt2", name=f"{pref}nt2")
                nc.gpsimd.tensor_tensor(nt2, st["nt1"], x, AL.subtract)
                nc.gpsimd.tensor_scalar(nt2, nt2, -CLAMP_HI, 0.0, AL.max,
                                        AL.min)
                st["nt2"] = nt2



            def emit_final(rd):
                t, grp = rd
                act_fin = ACT_FINAL if rounds.index(rd) >= 2 else ()
                st = state[rd]
                nt3 = st["nt2"]
                s_all = sm.tile([P, GH], F32, tag="s", name=f"s_{t}_{grp}")
                ps = []
                for j in range(GH):
                    p = pp.tile([P, LK], BF16, tag="p",
                                name=f"p_{t}_{grp}_{j}")
                    ps.append(p)
                    if j in act_fin:
                        r3 = r1p.tile([P, LK], BF16, tag="r1",
                                      name=f"r3_{t}_{grp}_{j}")
                        nc.vector.tensor_scalar(r3, st["r0"][j],
                                                nt3[:, j:j + 1], 0.0,
                                                AL.add, AL.max)
                        nc.scalar.activation(p, r3, AF.Square,
                                             accum_out=s_all[:, j:j + 1])
                    else:
                        nc.vector._custom_dve(
                            ENTMAX_OP, out=p, in0=st["r0"][j],
                            s0=nt3[:, j:j + 1], s1=0.0,
                            accum_out=s_all[:, j:j + 1])
                sh = sm.tile([P, GH], F32, tag="sh", name=f"sh_{t}_{grp}")
                nc.gpsimd.tensor_scalar(sh, s_all, float(H), EPS,
                                        AL.mult, AL.add)
                cinv = sm.tile([P, GH], F32, tag="cinv", name=f"c_{t}_{grp}")
                nc.vector.reciprocal(cinv, sh)
                acc = accs[t]
                for j in range(GH):
                    h = grp * GH + j
                    diag = dgp.tile([P, P], BF16, tag="diag", name="diag")
                    nc.gpsimd.tensor_tensor(
                        diag, ident, cinv[:, j:j + 1].to_broadcast([P, P]),
                        AL.mult)
                    for cw, co in zip(CCS, CCO):
                        nc.tensor.matmul(
                            acc[:, co:co + cw], diag, ps[j][:, co:co + cw],
                            start=(h == 0), stop=(h == H - 1))
                if grp == NGRP - 1:
                    accf = accsb.tile([P, LK], F32, tag="accf",
                                      name=f"accf{t}")
                    nc.scalar.copy(accf, acc)
                    nc.sync.dma_start(
                        out=out_d[t * P:(t + 1) * P, :], in_=accf)
                del state[rd]

            # 3-deep software pipeline: at tick i, round i runs scores+eval0,
            # round i-1 runs quad1+eval1, round i-2 runs the tail
            # (quad2..final). Projections for m=0..2 precede tick 0 (round 0
            # = heads 0-5 needs them); m=3..5 are emitted inside tick 0.
            rounds = [(t, g) for t in range(NT) for g in range(NGRP)]
            n = len(rounds)
            for m in range(3):
                emit_projection(m)
                emit_scores_eval0(rounds[0], js=(2 * m, 2 * m + 1))
            for i in range(n + 1):
                if i >= 2:
                    emit_quad2(rounds[i - 2])
                if 1 <= i <= n:
                    emit_quad1(rounds[i - 1])
                if i < n and i > 0:
                    emit_scores_eval0(rounds[i])
                if i == 0:
                    for m in range(3, KD):
                        emit_projection(m)
                        emit_scores_eval0(rounds[1],
                                          js=(2 * (m - 3), 2 * (m - 3) + 1))
                if 1 <= i <= n:
                    emit_eval1(rounds[i - 1])
                if i >= 2:
                    emit_final(rounds[i - 2])
                if i == n:
                    emit_quad2(rounds[n - 1])
                    emit_final(rounds[n - 1])

    nc.compile()
    return nc


_CACHE = {}


def kernel(**inputs) -> np.ndarray:
    query = np.ascontiguousarray(inputs["query"], dtype=np.float32)
    key = np.ascontiguousarray(inputs["key"], dtype=np.float32)
    mask = np.asarray(inputs["mask"])
    wq_w = np.asarray(inputs["wq_w"], dtype=np.float32)
    wq_b = np.asarray(inputs["wq_b"], dtype=np.float32)
    wk_w = np.asarray(inputs["wk_w"], dtype=np.float32)
    wk_b = np.asarray(inputs["wk_b"], dtype=np.float32)

    counts = [(mask[b] != 0).sum() for b in range(B)]
    LK = max(LK_DEFAULT, ((int(max(counts)) + 63) // 64) * 64)
    if LK not in _CACHE:
        _CACHE[LK] = _build_program(LK)
    nc = _CACHE[LK]

    import ml_dtypes
    wqT = np.ascontiguousarray(wq_w.T.astype(ml_dtypes.bfloat16))
    wkT = np.ascontiguousarray(wk_w.T.astype(ml_dtypes.bfloat16))

    # host-side key compaction per batch: gather unmasked keys, pad to LK
    idxs = []
    key_c = np.zeros((B, LK, D), dtype=np.float32)  # noqa: F821
    maskmul = np.zeros((B, LK), dtype=np.float32)
    for b in range(B):
        idx = np.nonzero(mask[b] != 0)[0]
        assert len(idx) <= LK, f"unmasked keys {len(idx)} > LK={LK}"
        idxs.append(idx)
        key_c[b, :len(idx)] = key[b, idx]
        maskmul[b, :len(idx)] = 1.0

    in_maps = []
    for c in range(CORES):
        b = c // (CORES // B)
        r0 = (c % (CORES // B)) * QS
        in_maps.append({
            "q_slice": np.ascontiguousarray(
                query[b, r0:r0 + QS, :].astype(ml_dtypes.bfloat16)),
            "key_c": key_c[b].astype(ml_dtypes.bfloat16),
            "wqT": wqT,
            "wkT": wkT,
            "bq": wq_b,
            "bk": wk_b,
            "maskmul": maskmul[b],
        })

    from concourse.bass_utils import run_bass_kernel_spmd

    res = run_bass_kernel_spmd(nc, in_maps, core_ids=list(range(CORES)))

    out = np.zeros((B, L, L), dtype=np.float32)
    for c in range(CORES):
        b = c // (CORES // B)
        r0 = (c % (CORES // B)) * QS
        sl = res.results[c]["out_slice"]       # [QS, LK]
        idx = idxs[b]
        view = out[b, r0:r0 + QS]
        view[:, idx] = sl[:, :len(idx)]
    return out


if __name__ == "__main__":
    import reference as R
    inp = R.setup_inputs()
    o = kernel(**{k: np.asarray(v) for k, v in inp.items()})
    print("kernel out", o.shape, o.dtype)

